# revision 41
# baseline (speedup 1.0000x reference)
"""EfficientAttention (linear attention) Trainium2 Bass kernel.

Computes, per batch b:
    q_n = softmax(q[b], axis=-1)        # over feature dim D=64
    k_n = softmax(k[b], axis=-1)
    ctx = k_n^T @ v[b]                  # [D, D]
    out[b] = q_n @ ctx                  # [N, D]

Sharding: batch dim (32) split across 8 cores, 4 batches per core.

Design notes (per core):
- I/O dtype is fp16: the host casts q/k/v fp32 -> fp16 before upload and
  upcasts o fp16 -> fp32 after download (rel err ~1e-3 vs the 2e-2 gate).
  This halves HBM traffic (67.1 MB -> 33.6 MB per core), which is the
  roofline for this memory-bound problem (~358 GB/s/core).
- DMA: 512 KB loads/stores ([128 partitions, 4 KB contiguous per partition];
  rows interleaved so partition p holds rows n0+32p .. n0+32p+31). Loads
  are prefetched two blocks ahead of their consumers.
- K/V pass: exp(k) on ACT (fp32), row-sums + reciprocal on DVE,
  normalize on gpsimd (otherwise idle), PE accumulates ctx[64,64] over N.
- ctx epilogue: block-diagonal stacked ctxa [128, 130] fp16
  (rows 0:64 = [ctx | 1 | 0], rows 64:128 = [0 | ctx | 1]) so one K=128
  matmul computes two packed row-tiles (cols 0:65 and 65:130, col 64/129
  = row sums via the ones columns).
- Q pass: PE-transpose raw q pairs [128, 2x64] -> PSUM [128,8,128] fp16,
  ACT exp PSUM->SBUF (fused evict + exp), per-slot matmuls vs ctxa into a
  2-bank PSUM tile [128, 2, 512] with slots padded to a uniform 256-float
  stride (slot w at offset 256*w), so the divide is ONE bulk reciprocal +
  ONE [128,4,2,64] DVE multiply per 1024 rows instead of 128 tiny ops per
  core (walrus emits one LDWEIGHTS per matmul - no dedupe - so a separate
  sums matmul would cost PE an extra 14 us; ones-columns are free).
- Fully software-pipelined 5-phase schedule (see the tick loop): the kv
  chain runs one tick ahead of its ctx matmuls, and q transpose+exp runs
  a full batch (NBLK ticks) ahead of its mm1/divide, so every
  cross-engine dependency is satisfied >= 1 tick early and each engine's
  in-order stream always has ready work. Each matmul writes a full row
  group (matmuls with alternating row groups on one PSUM bank lock up
  the device).
"""

import numpy as np

import concourse.bass as bass
import concourse.mybir as mybir
import concourse.tile as tile
from concourse import bacc
from concourse.bass_utils import run_bass_kernel_spmd

B, N, D = 32, 16384, 64
NCORES = 8
BPC = B // NCORES  # batches per core
LOAD = 4096  # rows per DMA (512 KB fp16)
LT = LOAD // 128  # row-tile slots per load (32)
NBLK = N // LOAD  # load blocks per batch (4)
F32 = mybir.dt.float32
F16 = mybir.dt.float16
EXP = mybir.ActivationFunctionType.Exp


def build_bass():
    nc = bacc.Bacc("TRN2", target_bir_lowering=False, debug=False)
    q = nc.dram_tensor("q", [BPC, N, D], F16, kind="ExternalInput").ap()
    k = nc.dram_tensor("k", [BPC, N, D], F16, kind="ExternalInput").ap()
    v = nc.dram_tensor("v", [BPC, N, D], F16, kind="ExternalInput").ap()
    o = nc.dram_tensor("o", [BPC, N, D], F16, kind="ExternalOutput").ap()

    def blk2(t, b, n0):
        # 8192-row (1 MB) load group: partition p holds rows n0+64p..n0+64p+63
        return t[b, n0 : n0 + 2 * LOAD, :].rearrange("(p t) d -> p t d", p=128)

    def oblk(b, i):
        # store view for compute block i: slots base..base+LT of its group,
        # mirroring the q-load row mapping (which rows land in which block
        # is irrelevant to the math; load/store mappings just must agree).
        n0 = (i // 2) * 2 * LOAD
        base = (i % 2) * LT
        return blk2(o, b, n0)[:, base : base + LT, :]

    with tile.TileContext(nc) as tc:
        with (
            tc.tile_pool(name="consts", bufs=1) as consts,
            tc.tile_pool(name="io", bufs=2) as io,
            tc.tile_pool(name="work", bufs=3) as work,
            tc.tile_pool(name="ctxp", bufs=2) as ctxp,
            tc.tile_pool(name="ps_t", bufs=2, space="PSUM") as ps_t,
            tc.tile_pool(name="ps_o", bufs=2, space="PSUM") as ps_o,
            tc.tile_pool(name="ps_c", bufs=2, space="PSUM") as ps_c,
        ):
            from concourse.masks import make_identity

            ident = consts.tile([128, 128], F16)
            make_identity(nc, ident)

            ctx_ps = {}
            kv_queue = []
            q_queue = []

            def load_kv_block(b, i):
                # 1 MB DMAs: blocks i (even) and i+1 load together; the two
                # compute blocks are slot-halves of one double-size tile.
                if i % 2 == 1:
                    return
                n0 = i * LOAD
                k_sb = io.tile([128, 2 * LT, 64], F16, tag="k_sb", bufs=3)
                v_sb = io.tile([128, 2 * LT, 64], F16, tag="v_sb", bufs=3)
                nc.sync.dma_start(out=k_sb, in_=blk2(k, b, n0))
                nc.sync.dma_start(out=v_sb, in_=blk2(v, b, n0))
                kv_queue.append((k_sb, 0, v_sb))
                kv_queue.append((k_sb, LT, v_sb))

            def load_q_block(b, i):
                if i % 2 == 1:
                    return
                q_sb = io.tile(
                    [128, 2 * LT, 64], F16, tag="q_sb", bufs=3, name="q_sb"
                )
                nc.sync.dma_start(out=q_sb, in_=blk2(q, b, i * LOAD))
                q_queue.append((q_sb, 0))
                q_queue.append((q_sb, LT))

            HLT = LT // 2  # half-block slots (16)
            ekn_queue = []
            eqT_queue = []

            def emit_kv_chain(b, i):
                # exp(k) -> row sums -> reciprocal -> normalize (in halves);
                # runs one tick ahead of the matmuls that consume ekn.
                k_sb, base, v_sb = kv_queue.pop(0)
                ekns = []
                for h in range(2):
                    s0 = base + h * HLT
                    ek = work.tile([128, HLT, 64], F32, tag="ek", bufs=4)
                    nc.scalar.activation(ek, k_sb[:, s0 : s0 + HLT, :], EXP)
                    ks = work.tile([128, HLT, 1], F32, tag="ks", bufs=4)
                    nc.vector.reduce_sum(out=ks, in_=ek, axis=mybir.AxisListType.X)
                    ksr = work.tile([128, HLT, 1], F32, tag="ksr", bufs=4)
                    nc.vector.reciprocal(ksr, ks)
                    ekn = work.tile([128, HLT, 64], F16, tag="ekn", bufs=4)
                    nc.gpsimd.tensor_mul(ekn, ek, ksr[:].to_broadcast((128, HLT, 64)))
                    ekns.append(ekn)
                ekn_queue.append((b, i, v_sb, base, ekns))

            def emit_kv_mm_half(h):
                b, i, v_sb, base, ekns = ekn_queue[0]
                ekn = ekns[h]
                s0 = base + h * HLT
                for t in range(HLT):
                    nc.tensor.matmul(
                        ctx_ps[b],
                        ekn[:, t, :],
                        v_sb[:, s0 + t, :],
                        start=(i == 0 and h == 0 and t == 0),
                        stop=(i == NBLK - 1 and h == 1 and t == HLT - 1),
                    )
                if h == 1:
                    ekn_queue.pop(0)
                    return (b, i)
                return None

            def emit_ctx_epilogue(b):
                ctxa = ctxp.tile([128, 130], F16, tag="ctxa")
                nc.vector.memset(ctxa, 0.0)
                nc.vector.tensor_copy(ctxa[0:64, 0:64], ctx_ps[b])
                nc.vector.memset(ctxa[0:64, 64:65], 1.0)
                nc.scalar.dma_start(out=ctxa[64:128, 65:130], in_=ctxa[0:64, 0:65])
                return ctxa

            def q_transposes(q_sb, base, h):
                tp_ps = ps_t.tile([128, 8, 128], F16, tag="tp_ps")
                for u in range(8):
                    s0 = base + 16 * h + 2 * u
                    nc.tensor.transpose(
                        tp_ps[:, u, :],
                        q_sb[:, s0 : s0 + 2, :].rearrange("p t d -> p (t d)"),
                        ident,
                    )
                return tp_ps

            def q_transp_exp(b, i):
                # transpose + exp of q(b, i); runs NBLK ticks ahead of the
                # mm1 stage (it does not need ctx), buffering eqT in SBUF.
                q_sb, base = q_queue.pop(0)
                eqs = []
                for h in range(2):
                    tp_ps = q_transposes(q_sb, base, h)
                    eqT = work.tile([128, 8, 128], F16, tag="eqT", bufs=10)
                    nc.scalar.activation(eqT, tp_ps, EXP)
                    eqs.append(eqT)
                eqT_queue.append(eqs)

            def q_mm_div(h, eqT, ctxa, out_sb):
                for g in range(2):
                        # 2-bank PSUM tile; slot w at offset 256*w (uniform
                        # stride across banks, 130 of 256 used) so the
                        # divide APs stay 4D (TENSOR3D encoding limit).
                        o_ps = ps_o.tile([128, 2, 512], F32, tag="o_ps")
                        for w in range(4):
                            u = 4 * g + w
                            nc.tensor.matmul(
                                o_ps[:, w // 2, 256 * (w % 2) : 256 * (w % 2) + 130],
                                eqT[:, u, :],
                                ctxa,
                                start=True,
                                stop=True,
                            )
                        opb = o_ps[:]
                        pdim = opb.ap[0]
                        rsr = work.tile([128, 4, 2, 1], F32, tag="rsr")
                        rs_ap = bass.AP(
                            tensor=opb.tensor,
                            offset=opb.offset + 64,
                            ap=[pdim, [256, 4], [65, 2], [1, 1]],
                        )
                        nc.vector.reciprocal(rsr, rs_ap)
                        vals_ap = bass.AP(
                            tensor=opb.tensor,
                            offset=opb.offset,
                            ap=[pdim, [256, 4], [65, 2], [1, 64]],
                        )
                        t0 = 16 * h + 8 * g
                        out_view = out_sb[:, t0 : t0 + 8, :].rearrange(
                            "p (s t) d -> p s t d", s=4
                        )
                        nc.vector.tensor_mul(
                            out_view,
                            vals_ap,
                            rsr[:].to_broadcast((128, 4, 2, 64)),
                        )

            # Fully software-pipelined schedule over global ticks T = 0..19
            # (5 phases x NBLK). Stage lags decouple every cross-engine chain:
            #   loads(T)      : kv + q block T+2's data
            #   chain(T)      : kv chain for block T        (ACT/DVE/Pool)
            #   transp/exp(T) : q(b,i) with 4b+i == T       (PE/ACT)
            #   kv mm(T)      : block T-1's ctx matmuls     (PE)
            #   mm1/div(T)    : q(b,i) with 4(b+1)+i == T   (PE/DVE) + store
            # so each consumer's inputs were produced >= 1 tick earlier and
            # every engine's in-order stream always has ready work.
            NT = (BPC + 1) * NBLK
            ctxa = None

            def blk_of(t):
                return (t // NBLK, t % NBLK)

            ctx_ps[0] = ps_c.tile([64, 64], F32, tag="ctx_ps", name="ctx_ps")
            for t in range(2):
                load_kv_block(*blk_of(t))
                load_q_block(*blk_of(t))
            for T in range(NT):
                # loads (lead 2 over chain/transp)
                if T + 2 < BPC * NBLK:
                    load_kv_block(*blk_of(T + 2))
                    load_q_block(*blk_of(T + 2))
                # kv chain for block T
                if T < BPC * NBLK:
                    cb, ci = blk_of(T)
                    if ci == 0 and cb > 0:
                        ctx_ps[cb] = ps_c.tile(
                            [64, 64], F32, tag="ctx_ps", name="ctx_ps"
                        )
                    emit_kv_chain(cb, ci)
                # q transpose+exp for block T
                if T < BPC * NBLK:
                    q_transp_exp(*blk_of(T))
                # interleave: kv mms (block T-1) with mm1/div (block T-NBLK)
                qT = T - NBLK
                out_sb = None
                if qT >= 0:
                    out_sb = io.tile([128, LT, 64], F16, tag="out_sb", bufs=4)
                have_mm = T >= 1 and T - 1 < BPC * NBLK
                # At batch boundaries the kv block being matmul'd is the last
                # of its batch: its epilogue must precede this tick's mm1s.
                boundary = have_mm and blk_of(T - 1)[1] == NBLK - 1
                if boundary:
                    for h in range(2):
                        done = emit_kv_mm_half(h)
                    ctxa = emit_ctx_epilogue(done[0])
                # Per half: kv matmuls lead, then mm1+divide. (Putting the
                # mm1s first was measured slower: the second half's mm1s
                # stall on PSUM banks still being divided, blocking the kv
                # matmuls queued behind them in the PE's in-order stream.)
                for h in range(2):
                    if have_mm and not boundary:
                        emit_kv_mm_half(h)
                    if qT >= 0:
                        q_mm_div(h, eqT_queue[0][h], ctxa, out_sb)
                if qT >= 0:
                    eqT_queue.pop(0)
                    qb, qi = blk_of(qT)
                    last = qT == BPC * NBLK - 1
                    if last:
                        for h in range(2):
                            nc.scalar.dma_start(
                                out=oblk(qb, qi)[:, 16 * h : 16 * h + 16, :],
                                in_=out_sb[:, 16 * h : 16 * h + 16, :],
                            )
                    else:
                        nc.scalar.dma_start(out=oblk(qb, qi), in_=out_sb)

    nc.compile()
    return nc


_NC_CACHE = None


def kernel(q: np.ndarray, k: np.ndarray, v: np.ndarray) -> np.ndarray:
    global _NC_CACHE
    if _NC_CACHE is None:
        _NC_CACHE = build_bass()
    nc = _NC_CACHE
    q = np.ascontiguousarray(np.asarray(q), dtype=np.float16)
    k = np.ascontiguousarray(np.asarray(k), dtype=np.float16)
    v = np.ascontiguousarray(np.asarray(v), dtype=np.float16)
    in_maps = [
        {
            "q": q[i * BPC : (i + 1) * BPC],
            "k": k[i * BPC : (i + 1) * BPC],
            "v": v[i * BPC : (i + 1) * BPC],
        }
        for i in range(NCORES)
    ]
    res = run_bass_kernel_spmd(nc, in_maps, core_ids=list(range(NCORES)))
    return np.concatenate(
        [res.results[i]["o"] for i in range(NCORES)], axis=0
    ).astype(np.float32)
